# revision 1
# baseline (speedup 1.0000x reference)
"""AffinityPropagate Trainium2 kernel.

Reference computation (per batch element):
    k_d = softmax(guided_d, axis=channel)          d = 1,2,3 (dilations)
    repeat 8 times:
        o_d = sum_ch k_d[ch] * shift(x, offset(d, ch))
        x   = o_1*fuse[0] + o_2*fuse[1] + o_3*fuse[2]

Strategy: pure data parallel over the batch (8 batches -> 8 NeuronCores).
Per core, the three 9-tap dilated kernels are pre-fused with the fuse
weights into 25 distinct-offset weight fields (the three (0,0) taps
share one field) stored in fp16 in SBUF.  x is kept in a halo layout:
partition p owns image rows [4p, 4p+4), stored with 3 halo rows on each
side and 4 zero border columns on each side ([120, 10, 648] fp16).

Each iteration: per tap, VectorE multiplies the weight field with a
shifted window of x (fp16, 2x perf mode, two taps per scratch tile to
halve PE semaphore overhead); TensorE accumulates the 25 product fields
into PSUM in fp32 via identity-stationary matmuls; ScalarE evacuates
PSUM back to the fp16 x buffer (directly to fp32 on the last
iteration).  Halo rows are rebuilt by TensorE with shift-by-one-
partition matmuls (SBUF->SBUF DMA measured ~12us for the same job) --
this also keeps the PE HAM clock warm across iteration boundaries.
(Odd-column-offset reads measure full 2x DVE rate despite the
documented 4B-alignment condition, so no shifted copy of x is needed.)

The guided tensors stream in at the measured ~160 GB/s DMA ceiling
(~210us); iteration-1 taps of each dilation are emitted between the
setup stages of the dilations so they execute under that DMA stream.
Guided-channel DMAs alternate between the sync (HWDGE) and gpsimd
(SWDGE) queue sets; x and fuse loads ride behind the first dilation's
channels since they are not needed until iteration-1 taps start.

Measured on one core (neuron-profile): ~528us total = ~200us DMA-bound
setup (weights + iteration-1 mostly hidden) + 7 x ~39us iterations
(DVE tap-multiply bound: 25 x 1.49us) + tail.  GpSimd stays idle:
concurrent GpSimd tensor ops hard-block DVE's shared SBUF port
(measured 1.5-3x DVE slowdown).
"""

import numpy as np

import concourse.bacc as bacc
import concourse.bass as bass
import concourse.mybir as mybir
from concourse.bass_utils import run_bass_kernel_spmd
from concourse.masks import make_identity
from concourse.tile import TileContext

H, W = 480, 640
P = 120          # partitions used (each owns R rows)
R = 4            # rows per partition
HALO = 3         # halo rows each side
CB = 4           # border cols each side (4 keeps packed reads 4B aligned)
ROWB = R + 2 * HALO          # 10 buffer rows per partition
COLB = W + 2 * CB            # 648 buffer cols
NFLAT = ROWB * COLB
PROP_TIME = 8
NCORES = 8

F16 = mybir.dt.float16
F32 = mybir.dt.float32

# taps run on GpSimd instead of DVE -- empty: concurrent GpSimd
# tensor_tensor hard-blocks DVE's shared SBUF port (measured 3x DVE
# slowdown), so GpSimd stays idle.
GP_TAPS = []
# DVE taps, ordered so halo-independent (dh == 0) taps come first
DVE_TAPS_EARLY = [(0, 0), (0, -2), (0, 2), (0, 1), (0, -1),
                  (0, 3), (0, -3)]


def _tap_table():
    """field_of[(dh, dw)] -> weight-field index (taps of all dilations)."""
    field_of = {(0, 0): 0}
    f = 1
    for d in (1, 2, 3):
        for ch in range(9):
            if ch == 4:
                continue
            dh = (ch // 3 - 1) * d
            dw = (ch % 3 - 1) * d
            field_of[(dh, dw)] = f
            f += 1
    assert f == 25
    return field_of


FIELD_OF = _tap_table()
ALL_TAPS = list(FIELD_OF.keys())
DVE_TAPS = DVE_TAPS_EARLY + [
    t for t in ALL_TAPS if t not in DVE_TAPS_EARLY and t not in GP_TAPS
]
assert len(DVE_TAPS) + len(GP_TAPS) == 25


def build_nc():
    nc = bacc.Bacc("TRN2", target_bir_lowering=False, debug=False)

    g_dram = [
        nc.dram_tensor(name, [9, H, W], F32, kind="ExternalInput")
        for name in ("guided1", "guided2", "guided3")
    ]
    fuse_dram = nc.dram_tensor("fuse", [3, H, W], F32, kind="ExternalInput")
    x_dram = nc.dram_tensor("x", [1, H, W], F32, kind="ExternalInput")
    out_dram = nc.dram_tensor("out", [1, H, W], F32, kind="ExternalOutput")

    # DRAM access patterns: partition p <- rows [4p, 4p+4) (one
    # contiguous 10KB descriptor per partition)
    def rows_ap(t, extra_off=0):
        return bass.AP(t, extra_off, [[R * W, P], [1, R * W]])

    uid = [0]

    def nm(pfx):
        uid[0] += 1
        return f"{pfx}{uid[0]}"

    with TileContext(nc) as tc:
        with (
            tc.tile_pool(name="const", bufs=1) as constp,
            tc.tile_pool(name="wpool", bufs=1) as wpool,
            tc.tile_pool(name="xmain", bufs=1) as xmain,
            tc.tile_pool(name="psit", bufs=1, space="PSUM") as psi,
        ):
            ident = constp.tile([P, P], F16)
            make_identity(nc, ident)
            # shift-by-one-partition matrices: S_up moves partition p-1's
            # data to p (top halo), S_dn the reverse
            S_up = constp.tile([P, P], F16, tag="sup")
            S_dn = constp.tile([P, P], F16, tag="sdn")
            for tile_, base in ((S_up, 1), (S_dn, -1)):
                nc.gpsimd.memset(tile_, 0.0)
                nc.gpsimd.affine_select(
                    out=tile_, in_=tile_,
                    compare_op=mybir.AluOpType.not_equal,
                    fill=1.0, base=base, pattern=[[-1, P]],
                    channel_multiplier=1,
                )

            wt = [wpool.tile([P, R, W], F16, tag=f"w{t}", name=f"w{t}")
                  for t in range(25)]
            XA = xmain.tile([P, ROWB, COLB], F16, tag="XA")
            XB = xmain.tile([P, ROWB, COLB], F16, tag="XB")

            mm_n = [0]

            def acc(ps, mflat, base, stop):
                first = mm_n[0] == 0
                for k in range(5):
                    nc.tensor.matmul(
                        out=ps[:, k * 512:(k + 1) * 512],
                        lhsT=ident,
                        rhs=mflat[:, base + k * 512:base + (k + 1) * 512],
                        start=first, stop=stop,
                    )
                    mm_n[0] += 1

            last_mf = [None]

            def emit_taps(taps, Xc, XS, ps, pool, group, stop_at_end,
                          psz=2):
                """Emit tap multiplies (psz per scratch tile) + PE
                accumulation."""
                pairs = [taps[i:i + psz] for i in range(0, len(taps), psz)]
                for pi, pair in enumerate(pairs):
                    m = pool.tile([P, len(pair), R, W], F16, tag="m",
                                  name=nm(f"m{group}_"))
                    for si, (dh, dw) in enumerate(pair):
                        win = Xc[:, HALO + dh:HALO + dh + R,
                                 CB + dw:CB + dw + W]
                        nc.vector.tensor_mul(
                            out=m[:, si, :, :], in0=wt[FIELD_OF[(dh, dw)]],
                            in1=win,
                        )
                    mf = m.rearrange("p s a b -> p (s a b)")
                    last_mf[0] = mf
                    for si in range(len(pair)):
                        acc(ps, mf, si * R * W,
                            stop_at_end and pi == len(pairs) - 1
                            and si == len(pair) - 1)

            HB3 = HALO * COLB      # 1944, halo bytes span per side

            def emit_tail(Xn, ps, last_iter, shiftp, XS=None):
                """Evacuate PSUM, rebuild halo rows via PE partition
                shifts (no slow SBUF->SBUF DMA), refill the shifted-x
                copy for the next iteration."""
                if last_iter:
                    return
                nc.scalar.copy(
                    out=Xn[:, HALO:HALO + R, CB:CB + W],
                    in_=ps.rearrange("p (a b) -> p a b", a=R),
                )
                Xn_f = Xn.rearrange("p a b -> p (a b)")
                for S, src0, dst0 in (
                    (S_up, (HALO + 1) * COLB, 0),
                    (S_dn, HALO * COLB, (R + HALO) * COLB),
                ):
                    for r0 in (0, 1024):
                        ln = min(1024, HB3 - r0)
                        psh = shiftp.tile([P, 1024], F32, tag="sh",
                                          name=nm("sh_"))
                        for c0 in range(0, ln, 512):
                            c1 = min(c0 + 512, ln)
                            nc.tensor.matmul(
                                out=psh[:, c0:c1], lhsT=S,
                                rhs=Xn_f[:, src0 + r0 + c0:src0 + r0 + c1],
                                start=True, stop=True,
                            )
                        nc.scalar.copy(
                            out=Xn_f[:, dst0 + r0:dst0 + r0 + ln],
                            in_=psh[:, 0:ln],
                        )

            # ---------------- setup + iteration 1 ----------------
            # Guided tensors stream in per dilation; iteration-1 taps of
            # each dilation run as soon as that dilation's weights are
            # ready, hiding compute under the ~240us DMA.
            ps0 = psi.tile([P, R * W], F32, tag="ps", name="ps_it0")
            with (
                tc.tile_pool(name="setup", bufs=2) as sp,
                tc.tile_pool(name="fusep", bufs=1) as fusep,
                tc.tile_pool(name="m0pool", bufs=2) as m0pool,
                tc.tile_pool(name="psst", bufs=1, space="PSUM") as psp,
            ):
                nc.vector.memset(XA, 0.0)
                nc.vector.memset(XB, 0.0)
                dma_engs = [nc.sync, nc.gpsimd]
                for d_idx in range(3):
                    d = d_idx + 1

                    def fld_of(ch):
                        return FIELD_OF[((ch // 3 - 1) * d, (ch % 3 - 1) * d)]

                    f16 = fusep.tile([P, R, W], F16, tag="f16",
                                     name=nm("f16_"))
                    e_c = fusep.tile([P, R, W], F16, tag="ec", name=nm("ec_"))
                    for ch in range(9):
                        g = sp.tile([P, R * W], F32, tag="g", name=nm("g_"))
                        dma_engs[ch % 2].dma_start(
                            out=g, in_=rows_ap(g_dram[d_idx], ch * H * W)
                        )
                        dest = e_c if ch == 4 else wt[fld_of(ch)]
                        nc.scalar.activation(
                            out=dest.rearrange("p a b -> p (a b)"), in_=g,
                            func=mybir.ActivationFunctionType.Exp,
                        )
                    f32t = sp.tile([P, R * W], F32, tag="g", name=nm("f32t_"))
                    nc.sync.dma_start(
                        out=f32t, in_=rows_ap(fuse_dram, d_idx * H * W)
                    )
                    nc.vector.tensor_copy(
                        out=f16, in_=f32t.rearrange("p (a b) -> p a b", a=R)
                    )
                    if d_idx == 0:
                        # x load rides behind dilation 1's channels --
                        # it's only needed once iteration-1 taps start
                        xs32 = sp.tile([P, R * W], F32, tag="g")
                        nc.sync.dma_start(out=xs32, in_=rows_ap(x_dram))
                        nc.vector.tensor_copy(
                            out=XA[:, HALO:HALO + R, CB:CB + W],
                            in_=xs32.rearrange("p (a b) -> p a b", a=R),
                        )
                        XA_f = XA.rearrange("p a b -> p (a b)")
                        nc.sync.dma_start(
                            out=XA_f[1:P, 0:HALO * COLB],
                            in_=XA_f[0:P - 1, R * COLB:(R + HALO) * COLB],
                        )
                        nc.sync.dma_start(
                            out=XA_f[0:P - 1, (R + HALO) * COLB:NFLAT],
                            in_=XA_f[1:P, HALO * COLB:2 * HALO * COLB],
                        )
                    # channel sums + 1/sum in halves (3 PSUM banks)
                    t_ = fusep.tile([P, R * W], F16, tag="t", name=nm("t_"))
                    f16f = f16.rearrange("p a b -> p (a b)")
                    HB = R * W // 2
                    for h0 in (0, HB):
                        pss = psp.tile([P, HB], F32, tag="pss",
                                       name=nm("pss_"))
                        for ch in range(9):
                            src = e_c if ch == 4 else wt[fld_of(ch)]
                            sf = src.rearrange("p a b -> p (a b)")
                            for c0 in range(0, HB, 512):
                                c1 = min(c0 + 512, HB)
                                nc.tensor.matmul(
                                    out=pss[:, c0:c1], lhsT=ident,
                                    rhs=sf[:, h0 + c0:h0 + c1],
                                    start=(ch == 0), stop=(ch == 8),
                                )
                        r = fusep.tile([P, HB], F32, tag="r", name=nm("r_"))
                        nc.vector.reciprocal_approx_fast(out=r, in_=pss)
                        nc.vector.tensor_mul(
                            out=t_[:, h0:h0 + HB], in0=f16f[:, h0:h0 + HB],
                            in1=r,
                        )
                    tv = t_.rearrange("p (a b) -> p a b", a=R)
                    for ch in range(9):
                        if ch == 4:
                            continue
                        wv = wt[fld_of(ch)]
                        nc.vector.tensor_mul(out=wv, in0=wv, in1=tv)
                    if d_idx == 0:
                        nc.vector.tensor_mul(out=wt[0], in0=e_c, in1=tv)
                    else:
                        nc.vector.tensor_mul(out=e_c, in0=e_c, in1=tv)
                        nc.vector.tensor_add(out=wt[0], in0=wt[0], in1=e_c)
                    # iteration-1 taps of this dilation (center last)
                    taps = [((ch // 3 - 1) * d, (ch % 3 - 1) * d)
                            for ch in range(9) if ch != 4]
                    if d_idx == 2:
                        taps.append((0, 0))
                    emit_taps(taps, XA, None, ps0, m0pool, f"i0d{d}",
                              stop_at_end=(d_idx == 2), psz=1)
                assert mm_n[0] == 125

            # ---------------- iterations 2..8 ----------------
            with (
                tc.tile_pool(name="xiter", bufs=1) as xiter,
                tc.tile_pool(name="shp", bufs=1, space="PSUM") as shiftp,
            ):
                with tc.tile_pool(name="mpool", bufs=4) as mpool:
                    emit_tail(XB, ps0, False, shiftp)

                    bufs = [XA, XB]
                    ps = ps0
                    for it in range(1, PROP_TIME):
                        Xc = bufs[it % 2]
                        Xn = bufs[(it + 1) % 2]
                        mm_n[0] = 0
                        ps = psi.tile([P, R * W], F32, tag="ps",
                                      name=nm("ps_"))
                        emit_taps(DVE_TAPS, Xc, None, ps, mpool, f"i{it}",
                                  stop_at_end=True)
                        assert mm_n[0] == 125
                        emit_tail(Xn, ps, it == PROP_TIME - 1, shiftp)

                with tc.tile_pool(name="stagep", bufs=1) as stagep:
                    stage = stagep.tile([P, R * W], F32)
                    nc.scalar.copy(out=stage, in_=ps)
                    nc.sync.dma_start(out=rows_ap(out_dram), in_=stage)

    nc.compile()
    return nc


_NC = None


def _get_nc():
    global _NC
    if _NC is None:
        _NC = build_nc()
    return _NC


def _in_maps(guided1, guided2, guided3, fuse, x):
    maps = []
    for b in range(NCORES):
        maps.append({
            "guided1": np.ascontiguousarray(guided1[b], dtype=np.float32),
            "guided2": np.ascontiguousarray(guided2[b], dtype=np.float32),
            "guided3": np.ascontiguousarray(guided3[b], dtype=np.float32),
            "fuse": np.ascontiguousarray(fuse[b], dtype=np.float32),
            "x": np.ascontiguousarray(x[b], dtype=np.float32),
        })
    return maps


def kernel(guided1, guided2, guided3, fuse, x):
    nc = _get_nc()
    res = run_bass_kernel_spmd(
        nc, _in_maps(guided1, guided2, guided3, fuse, x),
        core_ids=list(range(NCORES)),
    )
    return np.stack([res.results[b]["out"] for b in range(NCORES)], axis=0)


def kernel_profiled(guided1, guided2, guided3, fuse, x):
    """Returns (output, BassKernelResults) with trace enabled."""
    nc = _get_nc()
    res = run_bass_kernel_spmd(
        nc, _in_maps(guided1, guided2, guided3, fuse, x),
        core_ids=list(range(NCORES)), trace=True,
    )
    out = np.stack([res.results[b]["out"] for b in range(NCORES)], axis=0)
    return out, res

